# revision 11
# baseline (speedup 1.0000x reference)
"""Causal multi-head attention (B=2, S=2048, D=1024, H=16) on 8 TRN2 NeuronCores.

Sharding: core c handles batch b=c//4 and the 4 heads [4*(c%4), 4*(c%4)+4).
Each core computes its heads' Q/K/V projections, causal attention, and a
column-shard of the output projection; the host sums the 4 partials per batch
and adds bo.

On-device layout strategy (zero on-device transposes):
  - host supplies xT [D,S] and pre-transposed weights
  - qT,kT computed as [e,s] (head-dim on partitions), v as [s,e]
  - scores computed transposed: scoresT[k,q] = kT^T @ qT  (softmax over the
    partition dim; no max-subtraction needed -- scores are O(1) here)
  - softmax denominators via M=1 ones-matmuls (PE partition reduction)
  - AV: wvT[e,q] = v^T @ probsT  -> exactly the moving-operand layout the
    O-projection needs; output written transposed, un-transposed on host.
All matmuls run in float32r (full PE rate at N>=256, ~1e-4 rel err).
"""

import numpy as np

B, S, D, H = 2, 2048, 1024, 16
HD = D // H  # 64
NCORES = 8
P = 128
SB = 512          # s/q block size
NSB = S // SB     # 4
NDK = D // P      # 8
NKT_ALL = S // P  # 16

_CACHE = {}


def _build_nc():
    import concourse.bacc as bacc
    import concourse.mybir as mybir
    import concourse.tile as tile

    F32R, F32 = mybir.dt.float32r, mybir.dt.float32
    EXP = mybir.ActivationFunctionType.Exp
    ADD = mybir.AluOpType.add

    nc = bacc.Bacc(None)
    xT = nc.declare_dram_parameter("xT", [D, S], F32R, isOutput=False)
    wqk = nc.declare_dram_parameter("wqkT", [D, 512], F32R, isOutput=False)
    wv = nc.declare_dram_parameter("wvT", [D, 256], F32R, isOutput=False)
    wo = nc.declare_dram_parameter("woT", [256, D], F32R, isOutput=False)
    bqk = nc.declare_dram_parameter("bqk", [P, 4], F32, isOutput=False)
    bv = nc.declare_dram_parameter("bv", [1, 256], F32, isOutput=False)
    masks = nc.declare_dram_parameter("masks", [P, 4 * SB], F32, isOutput=False)
    outT = nc.declare_dram_parameter("outT", [D, S], F32, isOutput=True)

    with tile.TileContext(nc) as tc:
        with (
            tc.tile_pool(name="w", bufs=1) as wp,
            tc.tile_pool(name="x", bufs=2) as xp,
            tc.tile_pool(name="qk", bufs=1) as qkp,
            tc.tile_pool(name="pb", bufs=2) as pbp,
            tc.tile_pool(name="sm", bufs=2) as smp,
            tc.tile_pool(name="z", bufs=2) as zp,
            tc.tile_pool(name="o", bufs=3) as op_,
            tc.tile_pool(name="psS", bufs=1, space="PSUM") as psS,   # sc0+sc1 = 4 banks
            tc.tile_pool(name="psW", bufs=1, space="PSUM") as psW,   # wv0+wv1 = 2 banks
            tc.tile_pool(name="psG", bufs=2, space="PSUM") as psG,   # proj/oproj = 2 banks
        ):
            # ---- weights / constants ----
            wqk_sb = wp.tile([P, NDK, 512], F32R)
            nc.sync.dma_start(wqk_sb[:], wqk[:].rearrange("(dk p) m -> p dk m", p=P))
            wv_sb = wp.tile([P, NDK, 256], F32R)
            nc.sync.dma_start(wv_sb[:], wv[:].rearrange("(dk p) m -> p dk m", p=P))
            wo_sb = wp.tile([P, 2, D], F32R)
            nc.sync.dma_start(wo_sb[:], wo[:].rearrange("(k p) m -> p k m", p=P))
            bqk_sb = wp.tile([P, 4], F32)
            nc.sync.dma_start(bqk_sb[:], bqk[:])
            bv_sb = wp.tile([1, 256], F32)
            nc.sync.dma_start(bv_sb[:], bv[:])
            bv_bc = wp.tile([P, 256], F32)
            nc.gpsimd.partition_broadcast(bv_bc[:], bv_sb[:])
            mask_sb = wp.tile([P, 4, SB], F32)
            nc.sync.dma_start(mask_sb[:], masks[:].rearrange("p (t c) -> p t c", t=4))
            ones_sb = wp.tile([P, 1], F32R)
            nc.vector.memset(ones_sb[:].bitcast(F32), 1.0)

            # ---- persistent activations ----
            qT = [qkp.tile([P, S], F32R, tag=f"qT{p}", name=f"qT{p}") for p in range(2)]
            kT = [qkp.tile([P, S], F32R, tag=f"kT{p}", name=f"kT{p}") for p in range(2)]
            v_sb = qkp.tile([P, NKT_ALL, 4, HD + 1], F32R, tag="v")
            nc.vector.memset(v_sb[:, :, :, HD:HD + 1].bitcast(F32), 1.0)

            xT_r = xT[:].rearrange("(dk p) s -> p dk s", p=P)

            for blk in range(NSB):
                # ======== projections for s-block blk ========
                x_sb = xp.tile([P, NDK, SB], F32R, tag="x")
                nc.sync.dma_start(x_sb[:], xT_r[:, :, blk * SB:(blk + 1) * SB])
                for p in range(2):
                    for t in range(2):  # 0 = q, 1 = k
                        ps = psG.tile([P, SB], F32, tag="g")
                        c0 = 256 * t + 128 * p
                        for dk in range(NDK):
                            nc.tensor.matmul(
                                ps[:], wqk_sb[:, dk, c0:c0 + 128], x_sb[:, dk, :],
                                start=(dk == 0), stop=(dk == NDK - 1))
                        dst = (qT if t == 0 else kT)[p]
                        nc.vector.tensor_scalar_add(
                            dst[:, blk * SB:(blk + 1) * SB], ps[:],
                            bqk_sb[:, 2 * t + p: 2 * t + p + 1])
                for st in range(4):
                    kt = blk * 4 + st
                    psv = psG.tile([P, 256], F32, tag="g")
                    for dk in range(NDK):
                        nc.tensor.matmul(
                            psv[:], x_sb[:, dk, st * P:(st + 1) * P], wv_sb[:, dk, :],
                            start=(dk == 0), stop=(dk == NDK - 1))
                    nc.vector.tensor_tensor(
                        v_sb[:, kt, :, 0:HD],
                        psv[:].rearrange("p (h e) -> p h e", h=4),
                        bv_bc[:].rearrange("p (h e) -> p h e", h=4), ADD)

                # ======== attention for q-block j = blk ========
                j = blk
                NKT = 4 * (j + 1)
                NG = NKT // 2  # 2-ktile groups
                zT = zp.tile([P, 2, SB], F32R, tag="zT")
                for p in range(2):
                    wv_tiles = [psW.tile([P, SB], F32, tag=f"wv{half}", name=f"wv{half}")
                                for half in range(2)]
                    for g in range(NG):
                        for half in range(2):
                            h = 2 * p + half
                            base = 64 * half
                            sc = psS.tile([P, 2, SB], F32, tag=f"sc{half}")
                            pr = pbp.tile([P, 2, SB], F32R, tag=f"pr{half}")
                            for i in range(2):
                                kt = 2 * g + i
                                nc.tensor.matmul(
                                    sc[:, i, :],
                                    kT[p][base:base + 64, kt * P:(kt + 1) * P],
                                    qT[p][base:base + 64, j * SB:(j + 1) * SB],
                                    start=True, stop=True)
                                t = kt - (NKT - 4)
                                if t >= 0:
                                    nc.vector.tensor_tensor(
                                        sc[:, i, :], sc[:, i, :], mask_sb[:, t, :], ADD)
                            nc.scalar.activation(pr[:], sc[:], EXP)
                            # AV with fused ones-column: row 64 = softmax denom
                            for i in range(2):
                                kt = 2 * g + i
                                nc.tensor.matmul(
                                    wv_tiles[half][0:HD + 1, :],
                                    v_sb[:, kt, h, :], pr[:, i, :],
                                    start=(kt == 0), stop=(kt == NKT - 1))
                    # normalize: zT rows [64*half : 64*half+64] = wv[0:64] / wv[64]
                    # (partition_broadcast / reciprocal_approx require base-0
                    #  APs on HW; DVE copies DO relocate across partitions)
                    for half in range(2):
                        wv_ps = wv_tiles[half]
                        dn = smp.tile([1, SB], F32, tag="dn")
                        nc.vector.tensor_copy(dn[:], wv_ps[HD:HD + 1, :])
                        rb = smp.tile([P, SB], F32, tag="rb")
                        nc.gpsimd.partition_broadcast(rb[0:HD, :], dn[:])
                        rr = smp.tile([P, SB], F32, tag="rr")
                        rs = smp.tile([P, SB], F32, tag="rs")
                        nc.vector.reciprocal_approx_accurate(
                            rr[0:HD, :], rb[0:HD, :], rs[0:HD, :])
                        if half == 0:
                            nc.vector.tensor_tensor(
                                zT[0:HD, p, :], wv_ps[0:HD, :], rr[0:HD, :],
                                mybir.AluOpType.mult)
                        else:
                            zt_o = smp.tile([HD, SB], F32R, tag="zt_o")
                            nc.vector.tensor_tensor(
                                zt_o[:], wv_ps[0:HD, :], rr[0:HD, :],
                                mybir.AluOpType.mult)
                            nc.sync.dma_start(zT[HD:P, p, :], zt_o[:])
                # ======== output projection for q-block j ========
                for et in range(NDK):
                    po = psG.tile([P, SB], F32, tag="g")
                    for k2 in range(2):
                        nc.tensor.matmul(
                            po[:], wo_sb[:, k2, et * P:(et + 1) * P], zT[:, k2, :],
                            start=(k2 == 0), stop=(k2 == 1))
                    ot = op_.tile([P, SB], F32, tag="ot")
                    nc.any.tensor_copy(ot[:], po[:])
                    nc.sync.dma_start(
                        outT[:][et * P:(et + 1) * P, j * SB:(j + 1) * SB], ot[:])

    nc.compile()
    return nc


def _host_inputs(inputs, Wq, bq, Wk, bk, Wv, bv, Wo, bo):
    """Build the 8 per-core input maps."""
    scale = np.float32(1.0 / np.sqrt(HD))
    in_maps = []
    # causal masks for the 4 diagonal k-tiles of a q-block
    kk = np.arange(P)[:, None]
    qq = np.arange(SB)[None, :]
    m = np.zeros((P, 4 * SB), dtype=np.float32)
    for t in range(4):
        m[:, t * SB:(t + 1) * SB] = np.where(P * t + kk <= qq, 0.0, -1e30)
    for c in range(NCORES):
        b = c // 4
        hg = c % 4
        hs = slice(4 * hg, 4 * hg + 4)
        xTc = np.ascontiguousarray(np.asarray(inputs[b], np.float32).T)
        WqT = np.asarray(Wq[hs], np.float32).transpose(2, 0, 1).reshape(D, 256) * scale
        WkT = np.asarray(Wk[hs], np.float32).transpose(2, 0, 1).reshape(D, 256)
        WvT = np.asarray(Wv[hs], np.float32).transpose(2, 0, 1).reshape(D, 256)
        wqkT = np.ascontiguousarray(np.concatenate([WqT, WkT], axis=1))
        bq_c = np.asarray(bq[hs], np.float32).reshape(256) * scale
        bk_c = np.asarray(bk[hs], np.float32).reshape(256)
        bqk_c = np.stack([bq_c[0:128], bq_c[128:256], bk_c[0:128], bk_c[128:256]], axis=1)
        bv_c = np.asarray(bv[hs], np.float32).reshape(1, 256)
        woT = np.ascontiguousarray(np.asarray(Wo, np.float32)[:, 256 * hg:256 * (hg + 1)].T)
        in_maps.append({
            "xT": xTc, "wqkT": wqkT, "wvT": np.ascontiguousarray(WvT),
            "woT": woT, "bqk": np.ascontiguousarray(bqk_c), "bv": bv_c,
            "masks": m,
        })
    return in_maps


def kernel(inputs, Wq, bq, Wk, bk, Wv, bv, Wo, bo):
    from concourse.bass_utils import run_bass_kernel_spmd

    if "nc" not in _CACHE:
        _CACHE["nc"] = _build_nc()
    nc = _CACHE["nc"]
    in_maps = _host_inputs(inputs, Wq, bq, Wk, bk, Wv, bv, Wo, bo)
    res = run_bass_kernel_spmd(nc, in_maps, list(range(NCORES)))
    out = np.zeros((B, S, D), dtype=np.float32)
    for c in range(NCORES):
        out[c // 4] += res.results[c]["outT"].T
    out += np.asarray(bo, np.float32)[None, None, :]
    return out
